# revision 10
# baseline (speedup 1.0000x reference)
"""AGNNConv distributed Bass kernel for 8 TRN2 NeuronCores (v4).

out = (1+eps)*feat + h,  h[d] = sum_{e: dst_e=d} p_e * norm_feat[src_e]
with p_e = edge-softmax grouped by src.

Algebra (softmax max-subtraction dropped -- identity in exact math):
    w_e = exp(beta*ew_e)
    z_n = sum_{e: src_e=n} w_e
    q_e = w_e / (||feat_src_e|| * z_src_e)     # per-edge scalar
    h_d = sum_{e: dst_e=d} q_e * feat[src_e]
    out = (1+eps)*feat + h

The per-edge gather of source features is a pure permutation of input rows
-> done on HOST (no float math). Device receives contiguous per-edge streams:
  feat_edges [128, tet*D] f32   feat[src_e] rows in edge-slot order
  zpadE      [128, tet*K] f32   src's K-slot padded edge-weight row per edge
  ewp        [128, tet]   f32   this edge's weight
  bitp       [14, tet*128] bf16 dst-within-tile bit planes (one-hot trick)

v5 engine layout (v3 was Vector-bound; v4's 2-byte strided-DMA cast blew up
descriptor count):
  - msg stays FP32 end to end; mm2 runs fp32 (4 cyc/row, LDWEIGHTS hidden
    under the 213ns streams) -- no f32->bf16 cast pass exists at all.
  - q folded into msg rows on GpSimd (677 small 1-input tensor_scalar_mul).
  - one-hot st built 4 tiles per op from a full PSUM bank (plain is_eq /
    relu(M-6), no per-tile scale), alternating Vector 2 : Scalar 1.
  - 1/sqrt(ss) as exp(-0.5*ln(ss)) so every ScalarE ACT (Exp/Ln/Copy/Relu/
    Square) stays in one table set -- v3 paid 76 ACT_TABLE_LOADs (97us).
  - mm1 batches run one 4-batch ahead of mm2 so the PE never stalls on the
    st round-trip.
"""

import sys

sys.path.insert(0, "/opt/trn_rl_repo")

import numpy as np

N, E, D = 50000, 640000, 128
NCORES = 8
SH = N // NCORES            # 6250 dst nodes per core
HTILES = (SH + 127) // 128  # 49 dst tiles per core

PAD_EW = -80.0              # exp(beta*PAD_EW) == 0 (inside ACT LUT range)


def _host_prep(src, dst, edge_weight):
    """Index/layout prep only (no float math on tensor values)."""
    src = np.asarray(src).astype(np.int64)
    dst = np.asarray(dst).astype(np.int64)
    ew = np.asarray(edge_weight).astype(np.float32)

    # ---- per-node src-grouped edge-weight rows (for z), fixed K ----
    deg = np.bincount(src, minlength=N)
    K = int(deg.max())
    order = np.argsort(src, kind="stable")
    starts = np.zeros(N + 1, np.int64)
    np.cumsum(deg, out=starts[1:])
    slot = np.arange(E, dtype=np.int64) - starts[src[order]]
    zpad = np.full((N + 1, K), PAD_EW, np.float32)
    zpad[src[order], slot] = ew[order]
    # pad node (index N): slot0 = 0 -> z = 1 for pad edges (q ~ 0 safely)
    zpad[N, 0] = 0.0

    # ---- per-core edge grouping by dst tile ----
    owner = dst // SH
    dstl = dst - owner * SH
    dtile = dstl // 128
    dbit = dstl % 128

    counts = np.zeros((NCORES, HTILES), np.int64)
    np.add.at(counts, (owner, dtile), 1)
    net = (counts.max(axis=0) + 127) // 128  # [HTILES] edge tiles per dst tile
    net = np.maximum(net, 1)
    tet = int(net.sum())
    seg_off = np.zeros(HTILES, np.int64)
    seg_off[1:] = np.cumsum(net)[:-1]
    EPAD = tet * 128

    core_idx = []
    for c in range(NCORES):
        m = np.nonzero(owner == c)[0]
        key = dtile[m]
        korder = np.argsort(key, kind="stable")
        me = m[korder]
        keys = key[korder]
        kb = np.r_[0, np.nonzero(np.diff(keys))[0] + 1]
        sf = np.zeros(len(keys), np.int64)
        sf[kb] = kb
        np.maximum.accumulate(sf, out=sf)
        within = np.arange(len(keys)) - sf
        pos = seg_off[keys] * 128 + within

        src_pad = np.full(EPAD, N, np.int64)      # pad edges read node N
        ewp = np.full(EPAD, PAD_EW, np.float32)
        bits = np.zeros(EPAD, np.int64)
        src_pad[pos] = src[me]
        ewp[pos] = ew[me]
        bits[pos] = dbit[me]

        bp = np.zeros((14, EPAD), np.float32)
        for b in range(7):
            bb = (bits >> b) & 1
            bp[2 * b + 1] = bb
            bp[2 * b] = 1 - bb
        core_idx.append((src_pad, ewp.reshape(tet, 128).T.copy(), bp))

    import ml_dtypes

    nb = np.zeros((14, 128), np.float32)
    nn = np.arange(128)
    for b in range(7):
        bb = (nn >> b) & 1
        nb[2 * b + 1] = bb
        nb[2 * b] = 1 - bb
    nb = nb.astype(ml_dtypes.bfloat16)

    return zpad, core_idx, nb, net, K


_COMPILED = {}


def _build(net, K):
    import concourse.bass as bass
    import concourse.bacc as bacc
    from concourse import mybir, tile

    f32 = mybir.dt.float32
    bf16 = mybir.dt.bfloat16
    u16 = mybir.dt.uint16
    AF = mybir.ActivationFunctionType
    ALU = mybir.AluOpType
    X = mybir.AxisListType.X

    tet = int(net.sum())
    nmax = int(net.max())

    nc = bacc.Bacc(None, debug=False)

    fe_ext = nc.dram_tensor("feat_edges", [128, tet * D], f32, kind="ExternalInput")
    ze_ext = nc.dram_tensor("zpadE", [128, tet * K], f32, kind="ExternalInput")
    ewp_ext = nc.dram_tensor("ewp", [128, tet], f32, kind="ExternalInput")
    bitp_ext = nc.dram_tensor("bitp", [14, tet * 128], bf16, kind="ExternalInput")
    nbits_ext = nc.dram_tensor("nbits", [14, 128], bf16, kind="ExternalInput")
    featmy_ext = nc.dram_tensor("feat_my", [SH, D], f32, kind="ExternalInput")
    beta_ext = nc.dram_tensor("beta", [1, 1], f32, kind="ExternalInput")
    eps_ext = nc.dram_tensor("eps", [1, 1], f32, kind="ExternalInput")
    out_ext = nc.dram_tensor("out", [SH, D], f32, kind="ExternalOutput")

    with tile.TileContext(nc) as tc:
        with (
            tc.tile_pool(name="persist", bufs=1) as pp,
            tc.tile_pool(name="msgp", bufs=2) as mpool,
            tc.tile_pool(name="msgq", bufs=2) as mqpool,
            tc.tile_pool(name="sqp", bufs=2) as qpool,
            tc.tile_pool(name="zp", bufs=2) as zpool,
            tc.tile_pool(name="small", bufs=3) as spool,
            tc.tile_pool(name="bitpool", bufs=2) as bpool,
            tc.tile_pool(name="stp", bufs=3) as stpool,
            tc.tile_pool(name="outp", bufs=3) as opool,
            tc.tile_pool(name="mpsum", bufs=3, space="PSUM") as mpsum,
            tc.tile_pool(name="hpsum", bufs=2, space="PSUM") as hpsum,
        ):
            # ---------- scalars ----------
            beta_s = pp.tile([1, 1], f32, tag="beta_s")
            eps_s = pp.tile([1, 1], f32, tag="eps_s")
            nc.sync.dma_start(out=beta_s[:], in_=beta_ext[:])
            nc.sync.dma_start(out=eps_s[:], in_=eps_ext[:])
            beta_b = pp.tile([128, 1], f32, tag="beta_b")
            ep1_b = pp.tile([128, 1], f32, tag="ep1_b")
            nc.gpsimd.partition_broadcast(beta_b[:], beta_s[:])
            nc.gpsimd.partition_broadcast(ep1_b[:], eps_s[:])
            nc.vector.tensor_scalar_add(ep1_b[:], ep1_b[:], 1.0)
            neg6 = pp.tile([128, 1], f32, tag="neg6")
            nc.vector.memset(neg6[:], -6.0)

            # ---------- global per-edge weight w = exp(beta*ew) ----------
            wv = pp.tile([128, tet], f32, tag="wv")
            nc.sync.dma_start(out=wv[:], in_=ewp_ext[:])
            nc.scalar.activation(wv[:], wv[:], AF.Exp, scale=beta_b[:])

            nbits = pp.tile([14, 128], bf16, tag="nbits")
            nc.sync.dma_start(out=nbits[:], in_=nbits_ext[:])

            # ---------- main loop over dst tiles ----------
            T = 0
            for i in range(HTILES):
                nh = int(net[i])
                nb4 = (nh + 3) // 4
                rows = min(128, SH - i * 128)

                msg = mpool.tile([128, nmax, D], f32, tag="msg")
                nc.sync.dma_start(
                    out=msg[:, :nh, :].rearrange("p a b -> p (a b)"),
                    in_=fe_ext[:, T * D : (T + nh) * D],
                )
                zrow = zpool.tile([128, nmax, K], f32, tag="zrow")
                nc.sync.dma_start(
                    out=zrow[:, :nh, :].rearrange("p a b -> p (a b)"),
                    in_=ze_ext[:, T * K : (T + nh) * K],
                )
                bp = bpool.tile([14, nmax, 128], bf16, tag="bp")
                nc.sync.dma_start(
                    out=bp[:, :nh, :].rearrange("p a b -> p (a b)"),
                    in_=bitp_ext[:, T * 128 : (T + nh) * 128],
                )

                # row sumsq -> 1/||row|| = exp(-0.5*ln(ss))
                sq = qpool.tile([128, nmax, D], bf16, tag="sq")
                nc.scalar.activation(
                    sq[:, :nh, :].rearrange("p a b -> p (a b)"),
                    msg[:, :nh, :].rearrange("p a b -> p (a b)"),
                    AF.Square,
                )
                ss = spool.tile([128, nmax], f32, tag="ss")
                nc.vector.tensor_reduce(ss[:, :nh], sq[:, :nh, :], X, ALU.add)
                # pad rows are all-zero: clamp ss to avoid ln(0)
                nc.vector.tensor_scalar_max(ss[:, :nh], ss[:, :nh], 1e-12)
                lns = spool.tile([128, nmax], f32, tag="lns")
                nc.scalar.activation(lns[:, :nh], ss[:, :nh], AF.Ln)
                rr = spool.tile([128, nmax], f32, tag="rr")
                nc.scalar.activation(rr[:, :nh], lns[:, :nh], AF.Exp, scale=-0.5)

                # z = sum exp(beta * zrow)  (pad rows have slot0=0 -> z>=1)
                zx = zpool.tile([128, nmax, K], bf16, tag="zx")
                nc.scalar.activation(
                    zx[:, :nh, :].rearrange("p a b -> p (a b)"),
                    zrow[:, :nh, :].rearrange("p a b -> p (a b)"),
                    AF.Exp,
                    scale=beta_b[:],
                )
                zs = spool.tile([128, nmax], f32, tag="zs")
                nc.vector.tensor_reduce(zs[:, :nh], zx[:, :nh, :], X, ALU.add)
                zrec = spool.tile([128, nmax], f32, tag="zrec")
                nc.vector.reciprocal(zrec[:, :nh], zs[:, :nh])

                # q = w * zrec * rr
                q = spool.tile([128, nmax], f32, tag="q")
                nc.vector.tensor_tensor(
                    q[:, :nh], zrec[:, :nh], wv[:, T : T + nh], ALU.mult
                )
                nc.vector.tensor_tensor(q[:, :nh], q[:, :nh], rr[:, :nh], ALU.mult)

                # q-scaled message rows (GpSimd, one small op per edge tile)
                msgq = mqpool.tile([128, nmax, D], f32, tag="msgq")
                for t in range(nh):
                    nc.gpsimd.tensor_scalar_mul(
                        msgq[:, t, :], msg[:, t, :], q[:, t : t + 1]
                    )

                # edge tiles: batched one-hot build, mm1 one batch ahead
                hp = hpsum.tile([128, D], f32, tag="hp")
                mp4s = [None] * nb4
                st4s = [None] * nb4

                def issue_batch(g):
                    t0 = 4 * g
                    w4 = min(4, nh - t0)
                    mp4 = mpsum.tile([128, 4, 128], f32, tag="mp4")
                    for j in range(w4):
                        nc.tensor.matmul(
                            mp4[:, j, :], bp[:, t0 + j, :], nbits[:],
                            start=True, stop=True,
                        )
                    st4 = stpool.tile([128, 4, 128], f32, tag="st4")
                    sflat = st4[:, :w4, :].rearrange("p a b -> p (a b)")
                    mflat = mp4[:, :w4, :].rearrange("p a b -> p (a b)")
                    if g % 3 == 2:
                        nc.scalar.activation(
                            sflat, mflat, AF.Relu, bias=neg6[:]
                        )
                    else:
                        nc.vector.tensor_scalar(
                            sflat, mflat, 7.0, None, op0=ALU.is_equal
                        )
                    mp4s[g] = mp4
                    st4s[g] = st4

                for g in range(nb4 + 1):
                    if g < nb4:
                        issue_batch(g)
                    gm = g - 1
                    if gm >= 0:
                        t0 = 4 * gm
                        for j in range(min(4, nh - t0)):
                            t = t0 + j
                            nc.tensor.matmul(
                                hp[:],
                                st4s[gm][:, j, :],
                                msgq[:, t, :],
                                start=(t == 0),
                                stop=(t == nh - 1),
                            )

                # out = hp + (1+eps)*feat_my
                ftm = opool.tile([128, D], f32, tag="ftm")
                nc.sync.dma_start(
                    out=ftm[:rows, :],
                    in_=featmy_ext[i * 128 : i * 128 + rows, :],
                )
                fts = opool.tile([128, D], f32, tag="fts")
                nc.scalar.activation(
                    fts[:rows, :], ftm[:rows, :], AF.Copy, scale=ep1_b[:rows, :]
                )
                ot = opool.tile([128, D], f32, tag="ot")
                nc.vector.tensor_tensor(
                    ot[:rows, :], fts[:rows, :], hp[:rows, :], ALU.add
                )
                nc.sync.dma_start(
                    out=out_ext[i * 128 : i * 128 + rows, :], in_=ot[:rows, :]
                )
                T += nh

    nc.finalize()
    return nc


def kernel(feat, edge_weight, src, dst, beta, eps):
    from concourse.bass_utils import run_bass_kernel_spmd

    feat = np.asarray(feat, dtype=np.float32)
    ew = np.asarray(edge_weight, dtype=np.float32)
    beta = np.asarray(beta, dtype=np.float32)
    eps = np.asarray(eps, dtype=np.float32)

    zpad, core_idx, nb, net, K = _host_prep(src, dst, ew)
    tet = int(net.sum())

    key = (K, tuple(int(x) for x in net))
    if key not in _COMPILED:
        _COMPILED[key] = _build(net, K)
    nc = _COMPILED[key]

    featP = np.vstack([feat, np.zeros((1, D), np.float32)])  # pad row = 0
    beta2 = beta.reshape(1, 1)
    eps2 = eps.reshape(1, 1)

    in_maps = []
    for c in range(NCORES):
        src_pad, ewd, bp = core_idx[c]
        fe = featP[src_pad].reshape(tet, 128, D).transpose(1, 0, 2)
        zE = zpad[src_pad].reshape(tet, 128, K).transpose(1, 0, 2)
        import ml_dtypes

        in_maps.append(
            {
                "feat_edges": np.ascontiguousarray(fe).reshape(128, tet * D),
                "zpadE": np.ascontiguousarray(zE).reshape(128, tet * K),
                "ewp": ewd,
                "bitp": np.ascontiguousarray(bp).astype(ml_dtypes.bfloat16),
                "nbits": nb,
                "feat_my": np.ascontiguousarray(feat[c * SH : (c + 1) * SH]),
                "beta": beta2,
                "eps": eps2,
            }
        )

    res = run_bass_kernel_spmd(nc, in_maps, core_ids=list(range(NCORES)))
    out = np.concatenate([res.results[c]["out"] for c in range(NCORES)], axis=0)
    return out.astype(np.float32)


# revision 13
# speedup vs baseline: 3.1596x; 3.1596x over previous
"""AGNNConv distributed Bass kernel for 8 TRN2 NeuronCores (v4).

out = (1+eps)*feat + h,  h[d] = sum_{e: dst_e=d} p_e * norm_feat[src_e]
with p_e = edge-softmax grouped by src.

Algebra (softmax max-subtraction dropped -- identity in exact math):
    w_e = exp(beta*ew_e)
    z_n = sum_{e: src_e=n} w_e
    q_e = w_e / (||feat_src_e|| * z_src_e)     # per-edge scalar
    h_d = sum_{e: dst_e=d} q_e * feat[src_e]
    out = (1+eps)*feat + h

The per-edge gather of source features is a pure permutation of input rows
-> done on HOST (no float math). Device receives contiguous per-edge streams:
  feat_edges [128, tet*D] f32   feat[src_e] rows in edge-slot order
  zpadE      [128, tet*K] f32   src's K-slot padded edge-weight row per edge
  ewp        [128, tet]   f32   this edge's weight
  bitp       [14, tet*128] bf16 dst-within-tile bit planes (one-hot trick)

v5 engine layout (v3 was Vector-bound; v4's 2-byte strided-DMA cast blew up
descriptor count):
  - msg stays FP32 end to end; mm2 runs fp32 (4 cyc/row, LDWEIGHTS hidden
    under the 213ns streams) -- no f32->bf16 cast pass exists at all.
  - q folded into msg rows on GpSimd (677 small 1-input tensor_scalar_mul).
  - one-hot st built 4 tiles per op from a full PSUM bank (plain is_eq /
    relu(M-6), no per-tile scale), alternating Vector 2 : Scalar 1.
  - 1/sqrt(ss) as exp(-0.5*ln(ss)) so every ScalarE ACT (Exp/Ln/Copy/Relu/
    Square) stays in one table set -- v3 paid 76 ACT_TABLE_LOADs (97us).
  - mm1 batches run one 4-batch ahead of mm2 so the PE never stalls on the
    st round-trip.
"""

import sys

sys.path.insert(0, "/opt/trn_rl_repo")

import numpy as np

N, E, D = 50000, 640000, 128
NCORES = 8
SH = N // NCORES            # 6250 dst nodes per core
HTILES = (SH + 127) // 128  # 49 dst tiles per core

PAD_EW = -80.0              # exp(beta*PAD_EW) == 0 (inside ACT LUT range)


def _host_prep(src, dst, edge_weight):
    """Index/layout prep only (no float math on tensor values)."""
    src = np.asarray(src).astype(np.int64)
    dst = np.asarray(dst).astype(np.int64)
    ew = np.asarray(edge_weight).astype(np.float32)

    # ---- per-node src-grouped edge-weight rows (for z), fixed K ----
    deg = np.bincount(src, minlength=N)
    K = int(deg.max())
    order = np.argsort(src, kind="stable")
    starts = np.zeros(N + 1, np.int64)
    np.cumsum(deg, out=starts[1:])
    slot = np.arange(E, dtype=np.int64) - starts[src[order]]
    zpad = np.full((N + 1, K), PAD_EW, np.float32)
    zpad[src[order], slot] = ew[order]
    # pad node (index N): slot0 = 0 -> z = 1 for pad edges (q ~ 0 safely)
    zpad[N, 0] = 0.0

    # ---- per-core edge grouping by dst tile ----
    owner = dst // SH
    dstl = dst - owner * SH
    dtile = dstl // 128
    dbit = dstl % 128

    counts = np.zeros((NCORES, HTILES), np.int64)
    np.add.at(counts, (owner, dtile), 1)
    net = (counts.max(axis=0) + 127) // 128  # [HTILES] edge tiles per dst tile
    net = np.maximum(net, 1)
    tet = int(net.sum())
    seg_off = np.zeros(HTILES, np.int64)
    seg_off[1:] = np.cumsum(net)[:-1]
    EPAD = tet * 128

    core_idx = []
    for c in range(NCORES):
        m = np.nonzero(owner == c)[0]
        key = dtile[m]
        korder = np.argsort(key, kind="stable")
        me = m[korder]
        keys = key[korder]
        kb = np.r_[0, np.nonzero(np.diff(keys))[0] + 1]
        sf = np.zeros(len(keys), np.int64)
        sf[kb] = kb
        np.maximum.accumulate(sf, out=sf)
        within = np.arange(len(keys)) - sf
        pos = seg_off[keys] * 128 + within

        src_pad = np.full(EPAD, N, np.int64)      # pad edges read node N
        ewp = np.full(EPAD, PAD_EW, np.float32)
        bits = np.zeros(EPAD, np.int64)
        src_pad[pos] = src[me]
        ewp[pos] = ew[me]
        bits[pos] = dbit[me]

        bp = np.zeros((14, EPAD), np.float32)
        for b in range(7):
            bb = (bits >> b) & 1
            bp[2 * b + 1] = bb
            bp[2 * b] = 1 - bb
        core_idx.append((src_pad, ewp.reshape(tet, 128).T.copy(), bp))

    import ml_dtypes

    nb = np.zeros((14, 128), np.float32)
    nn = np.arange(128)
    for b in range(7):
        bb = (nn >> b) & 1
        nb[2 * b + 1] = bb
        nb[2 * b] = 1 - bb
    nb = nb.astype(ml_dtypes.bfloat16)

    return zpad, core_idx, nb, net, K


_COMPILED = {}


def _build(net, K):
    import concourse.bass as bass
    import concourse.bacc as bacc
    from concourse import mybir, tile

    f32 = mybir.dt.float32
    bf16 = mybir.dt.bfloat16
    u16 = mybir.dt.uint16
    AF = mybir.ActivationFunctionType
    ALU = mybir.AluOpType
    X = mybir.AxisListType.X

    tet = int(net.sum())
    nmax = int(net.max())

    nc = bacc.Bacc(None, debug=False)

    fe_ext = nc.dram_tensor("feat_edges", [128, tet * D], f32, kind="ExternalInput")
    ze_ext = nc.dram_tensor("zpadE", [128, tet * K], f32, kind="ExternalInput")
    ewp_ext = nc.dram_tensor("ewp", [128, tet], f32, kind="ExternalInput")
    bitp_ext = nc.dram_tensor("bitp", [14, tet * 128], bf16, kind="ExternalInput")
    nbits_ext = nc.dram_tensor("nbits", [14, 128], bf16, kind="ExternalInput")
    featmy_ext = nc.dram_tensor("feat_my", [SH, D], f32, kind="ExternalInput")
    beta_ext = nc.dram_tensor("beta", [1, 1], f32, kind="ExternalInput")
    eps_ext = nc.dram_tensor("eps", [1, 1], f32, kind="ExternalInput")
    out_ext = nc.dram_tensor("out", [SH, D], f32, kind="ExternalOutput")

    with tile.TileContext(nc) as tc:
        with (
            tc.tile_pool(name="persist", bufs=1) as pp,
            tc.tile_pool(name="msgp", bufs=2) as mpool,
            tc.tile_pool(name="msgq", bufs=2) as mqpool,
            tc.tile_pool(name="sqp", bufs=2) as qpool,
            tc.tile_pool(name="zp", bufs=2) as zpool,
            tc.tile_pool(name="small", bufs=3) as spool,
            tc.tile_pool(name="bitpool", bufs=2) as bpool,
            tc.tile_pool(name="stp", bufs=3) as stpool,
            tc.tile_pool(name="outp", bufs=3) as opool,
            tc.tile_pool(name="mpsum", bufs=3, space="PSUM") as mpsum,
            tc.tile_pool(name="hpsum", bufs=2, space="PSUM") as hpsum,
        ):
            # ---------- scalars ----------
            beta_s = pp.tile([1, 1], f32, tag="beta_s")
            eps_s = pp.tile([1, 1], f32, tag="eps_s")
            nc.sync.dma_start(out=beta_s[:], in_=beta_ext[:])
            nc.sync.dma_start(out=eps_s[:], in_=eps_ext[:])
            beta_b = pp.tile([128, 1], f32, tag="beta_b")
            ep1_b = pp.tile([128, 1], f32, tag="ep1_b")
            nc.gpsimd.partition_broadcast(beta_b[:], beta_s[:])
            nc.gpsimd.partition_broadcast(ep1_b[:], eps_s[:])
            nc.vector.tensor_scalar_add(ep1_b[:], ep1_b[:], 1.0)
            neg6 = pp.tile([128, 1], f32, tag="neg6")
            nc.vector.memset(neg6[:], -6.0)

            # ---------- global per-edge weight w = exp(beta*ew) ----------
            wv = pp.tile([128, tet], f32, tag="wv")
            nc.sync.dma_start(out=wv[:], in_=ewp_ext[:])
            nc.scalar.activation(wv[:], wv[:], AF.Exp, scale=beta_b[:])

            nbits = pp.tile([14, 128], bf16, tag="nbits")
            nc.sync.dma_start(out=nbits[:], in_=nbits_ext[:])

            # ---------- main loop over dst tiles ----------
            T = 0
            for i in range(HTILES):
                nh = int(net[i])
                nb4 = (nh + 3) // 4
                rows = min(128, SH - i * 128)

                msg = mpool.tile([128, nmax, D], f32, tag="msg")
                nc.sync.dma_start(
                    out=msg[:, :nh, :].rearrange("p a b -> p (a b)"),
                    in_=fe_ext[:, T * D : (T + nh) * D],
                )
                zrow = zpool.tile([128, nmax, K], f32, tag="zrow")
                nc.sync.dma_start(
                    out=zrow[:, :nh, :].rearrange("p a b -> p (a b)"),
                    in_=ze_ext[:, T * K : (T + nh) * K],
                )
                bp = bpool.tile([14, nmax, 128], bf16, tag="bp")
                nc.sync.dma_start(
                    out=bp[:, :nh, :].rearrange("p a b -> p (a b)"),
                    in_=bitp_ext[:, T * 128 : (T + nh) * 128],
                )

                # row sumsq -> 1/||row|| = exp(-0.5*ln(ss))
                sq = qpool.tile([128, nmax, D], bf16, tag="sq")
                nc.gpsimd.tensor_tensor(
                    sq[:, :nh, :].rearrange("p a b -> p (a b)"),
                    msg[:, :nh, :].rearrange("p a b -> p (a b)"),
                    msg[:, :nh, :].rearrange("p a b -> p (a b)"),
                    ALU.mult,
                )
                ss = spool.tile([128, nmax], f32, tag="ss")
                nc.vector.tensor_reduce(ss[:, :nh], sq[:, :nh, :], X, ALU.add)
                # pad rows are all-zero: clamp ss to avoid ln(0)
                nc.vector.tensor_scalar_max(ss[:, :nh], ss[:, :nh], 1e-12)
                lns = spool.tile([128, nmax], f32, tag="lns")
                nc.scalar.activation(lns[:, :nh], ss[:, :nh], AF.Ln)
                rr = spool.tile([128, nmax], f32, tag="rr")
                nc.scalar.activation(rr[:, :nh], lns[:, :nh], AF.Exp, scale=-0.5)

                # z = sum exp(beta * zrow)  (pad rows have slot0=0 -> z>=1)
                zx = zpool.tile([128, nmax, K], bf16, tag="zx")
                nc.scalar.activation(
                    zx[:, :nh, :].rearrange("p a b -> p (a b)"),
                    zrow[:, :nh, :].rearrange("p a b -> p (a b)"),
                    AF.Exp,
                    scale=beta_b[:],
                )
                zs = spool.tile([128, nmax], f32, tag="zs")
                nc.vector.tensor_reduce(zs[:, :nh], zx[:, :nh, :], X, ALU.add)
                zrec = spool.tile([128, nmax], f32, tag="zrec")
                nc.vector.reciprocal(zrec[:, :nh], zs[:, :nh])

                # q = w * zrec * rr
                q = spool.tile([128, nmax], f32, tag="q")
                nc.vector.tensor_tensor(
                    q[:, :nh], zrec[:, :nh], wv[:, T : T + nh], ALU.mult
                )
                nc.vector.tensor_tensor(q[:, :nh], q[:, :nh], rr[:, :nh], ALU.mult)

                # q-scaled bf16 message rows: one Vector op per dst tile with
                # q broadcast along D (stride-0), fused f32->bf16 cast
                msgq = mqpool.tile([128, nmax, D], bf16, tag="msgq")
                qb = q[:, :nh].unsqueeze(2).broadcast_to([128, nh, D])
                nc.vector.tensor_tensor(
                    msgq[:, :nh, :], msg[:, :nh, :], qb, ALU.mult
                )

                # edge tiles: batched one-hot build, mm1 one batch ahead
                hp = hpsum.tile([128, D], f32, tag="hp")
                mp4s = [None] * nb4
                st4s = [None] * nb4

                def issue_batch(g):
                    t0 = 4 * g
                    w4 = min(4, nh - t0)
                    mp4 = mpsum.tile([128, 4, 128], f32, tag="mp4")
                    for j in range(w4):
                        nc.tensor.matmul(
                            mp4[:, j, :], bp[:, t0 + j, :], nbits[:],
                            start=True, stop=True,
                        )
                    st4 = stpool.tile([128, 4, 128], bf16, tag="st4")
                    sflat = st4[:, :w4, :].rearrange("p a b -> p (a b)")
                    mflat = mp4[:, :w4, :].rearrange("p a b -> p (a b)")
                    if g % 4 == 3:
                        nc.vector.tensor_scalar(
                            sflat, mflat, 7.0, None, op0=ALU.is_equal
                        )
                    else:
                        nc.scalar.activation(
                            sflat, mflat, AF.Relu, bias=neg6[:]
                        )
                    mp4s[g] = mp4
                    st4s[g] = st4

                for g in range(nb4 + 1):
                    if g < nb4:
                        issue_batch(g)
                    gm = g - 1
                    if gm >= 0:
                        t0 = 4 * gm
                        for j in range(min(4, nh - t0)):
                            t = t0 + j
                            nc.tensor.matmul(
                                hp[:],
                                st4s[gm][:, j, :],
                                msgq[:, t, :],
                                start=(t == 0),
                                stop=(t == nh - 1),
                            )

                # out = hp + (1+eps)*feat_my
                ftm = opool.tile([128, D], f32, tag="ftm")
                nc.sync.dma_start(
                    out=ftm[:rows, :],
                    in_=featmy_ext[i * 128 : i * 128 + rows, :],
                )
                fts = opool.tile([128, D], f32, tag="fts")
                nc.scalar.activation(
                    fts[:rows, :], ftm[:rows, :], AF.Copy, scale=ep1_b[:rows, :]
                )
                ot = opool.tile([128, D], f32, tag="ot")
                nc.vector.tensor_tensor(
                    ot[:rows, :], fts[:rows, :], hp[:rows, :], ALU.add
                )
                nc.sync.dma_start(
                    out=out_ext[i * 128 : i * 128 + rows, :], in_=ot[:rows, :]
                )
                T += nh

    nc.finalize()
    return nc


def kernel(feat, edge_weight, src, dst, beta, eps):
    from concourse.bass_utils import run_bass_kernel_spmd

    feat = np.asarray(feat, dtype=np.float32)
    ew = np.asarray(edge_weight, dtype=np.float32)
    beta = np.asarray(beta, dtype=np.float32)
    eps = np.asarray(eps, dtype=np.float32)

    zpad, core_idx, nb, net, K = _host_prep(src, dst, ew)
    tet = int(net.sum())

    key = (K, tuple(int(x) for x in net))
    if key not in _COMPILED:
        _COMPILED[key] = _build(net, K)
    nc = _COMPILED[key]

    featP = np.vstack([feat, np.zeros((1, D), np.float32)])  # pad row = 0
    beta2 = beta.reshape(1, 1)
    eps2 = eps.reshape(1, 1)

    in_maps = []
    for c in range(NCORES):
        src_pad, ewd, bp = core_idx[c]
        fe = featP[src_pad].reshape(tet, 128, D).transpose(1, 0, 2)
        zE = zpad[src_pad].reshape(tet, 128, K).transpose(1, 0, 2)
        import ml_dtypes

        in_maps.append(
            {
                "feat_edges": np.ascontiguousarray(fe).reshape(128, tet * D),
                "zpadE": np.ascontiguousarray(zE).reshape(128, tet * K),
                "ewp": ewd,
                "bitp": np.ascontiguousarray(bp).astype(ml_dtypes.bfloat16),
                "nbits": nb,
                "feat_my": np.ascontiguousarray(feat[c * SH : (c + 1) * SH]),
                "beta": beta2,
                "eps": eps2,
            }
        )

    res = run_bass_kernel_spmd(nc, in_maps, core_ids=list(range(NCORES)))
    out = np.concatenate([res.results[c]["out"] for c in range(NCORES)], axis=0)
    return out.astype(np.float32)


# revision 14
# speedup vs baseline: 3.6222x; 1.1464x over previous
"""AGNNConv distributed Bass kernel for 8 TRN2 NeuronCores (v4).

out = (1+eps)*feat + h,  h[d] = sum_{e: dst_e=d} p_e * norm_feat[src_e]
with p_e = edge-softmax grouped by src.

Algebra (softmax max-subtraction dropped -- identity in exact math):
    w_e = exp(beta*ew_e)
    z_n = sum_{e: src_e=n} w_e
    q_e = w_e / (||feat_src_e|| * z_src_e)     # per-edge scalar
    h_d = sum_{e: dst_e=d} q_e * feat[src_e]
    out = (1+eps)*feat + h

The per-edge gather of source features is a pure permutation of input rows
-> done on HOST (no float math). Device receives contiguous per-edge streams:
  feat_edges [128, tet*D] f32   feat[src_e] rows in edge-slot order
  zpadE      [128, tet*K] f32   src's K-slot padded edge-weight row per edge
  ewp        [128, tet]   f32   this edge's weight
  bitp       [14, tet*128] bf16 dst-within-tile bit planes (one-hot trick)

v5 engine layout (v3 was Vector-bound; v4's 2-byte strided-DMA cast blew up
descriptor count):
  - msg stays FP32 end to end; mm2 runs fp32 (4 cyc/row, LDWEIGHTS hidden
    under the 213ns streams) -- no f32->bf16 cast pass exists at all.
  - q folded into msg rows on GpSimd (677 small 1-input tensor_scalar_mul).
  - one-hot st built 4 tiles per op from a full PSUM bank (plain is_eq /
    relu(M-6), no per-tile scale), alternating Vector 2 : Scalar 1.
  - 1/sqrt(ss) as exp(-0.5*ln(ss)) so every ScalarE ACT (Exp/Ln/Copy/Relu/
    Square) stays in one table set -- v3 paid 76 ACT_TABLE_LOADs (97us).
  - mm1 batches run one 4-batch ahead of mm2 so the PE never stalls on the
    st round-trip.
"""

import sys

sys.path.insert(0, "/opt/trn_rl_repo")

import numpy as np

N, E, D = 50000, 640000, 128
NCORES = 8
SH = N // NCORES            # 6250 dst nodes per core
HTILES = (SH + 127) // 128  # 49 dst tiles per core

PAD_EW = -80.0              # exp(beta*PAD_EW) == 0 (inside ACT LUT range)


def _host_prep(src, dst, edge_weight):
    """Index/layout prep only (no float math on tensor values)."""
    src = np.asarray(src).astype(np.int64)
    dst = np.asarray(dst).astype(np.int64)
    ew = np.asarray(edge_weight).astype(np.float32)

    # ---- per-node src-grouped edge-weight rows (for z), fixed K ----
    deg = np.bincount(src, minlength=N)
    K = int(deg.max())
    order = np.argsort(src, kind="stable")
    starts = np.zeros(N + 1, np.int64)
    np.cumsum(deg, out=starts[1:])
    slot = np.arange(E, dtype=np.int64) - starts[src[order]]
    zpad = np.full((N + 1, K), PAD_EW, np.float32)
    zpad[src[order], slot] = ew[order]
    # pad node (index N): slot0 = 0 -> z = 1 for pad edges (q ~ 0 safely)
    zpad[N, 0] = 0.0

    # ---- per-core edge grouping by dst tile ----
    owner = dst // SH
    dstl = dst - owner * SH
    dtile = dstl // 128
    dbit = dstl % 128

    counts = np.zeros((NCORES, HTILES), np.int64)
    np.add.at(counts, (owner, dtile), 1)
    net = (counts.max(axis=0) + 127) // 128  # [HTILES] edge tiles per dst tile
    net = np.maximum(net, 1)
    tet = int(net.sum())
    seg_off = np.zeros(HTILES, np.int64)
    seg_off[1:] = np.cumsum(net)[:-1]
    EPAD = tet * 128

    core_idx = []
    for c in range(NCORES):
        m = np.nonzero(owner == c)[0]
        key = dtile[m]
        korder = np.argsort(key, kind="stable")
        me = m[korder]
        keys = key[korder]
        kb = np.r_[0, np.nonzero(np.diff(keys))[0] + 1]
        sf = np.zeros(len(keys), np.int64)
        sf[kb] = kb
        np.maximum.accumulate(sf, out=sf)
        within = np.arange(len(keys)) - sf
        pos = seg_off[keys] * 128 + within

        src_pad = np.full(EPAD, N, np.int64)      # pad edges read node N
        ewp = np.full(EPAD, PAD_EW, np.float32)
        bits = np.zeros(EPAD, np.int64)
        src_pad[pos] = src[me]
        ewp[pos] = ew[me]
        bits[pos] = dbit[me]

        bp = np.zeros((14, EPAD), np.float32)
        for b in range(7):
            bb = (bits >> b) & 1
            bp[2 * b + 1] = bb
            bp[2 * b] = 1 - bb
        core_idx.append((src_pad, ewp.reshape(tet, 128).T.copy(), bp))

    import ml_dtypes

    nb = np.zeros((14, 128), np.float32)
    nn = np.arange(128)
    for b in range(7):
        bb = (nn >> b) & 1
        nb[2 * b + 1] = bb
        nb[2 * b] = 1 - bb
    nb = nb.astype(ml_dtypes.bfloat16)

    return zpad, core_idx, nb, net, K


_COMPILED = {}


def _build(net, K):
    import concourse.bass as bass
    import concourse.bacc as bacc
    from concourse import mybir, tile

    f32 = mybir.dt.float32
    bf16 = mybir.dt.bfloat16
    u16 = mybir.dt.uint16
    AF = mybir.ActivationFunctionType
    ALU = mybir.AluOpType
    X = mybir.AxisListType.X

    tet = int(net.sum())
    nmax = int(net.max())

    nc = bacc.Bacc(None, debug=False)

    fe_ext = nc.dram_tensor("feat_edges", [128, tet * D], f32, kind="ExternalInput")
    ze_ext = nc.dram_tensor("zpadE", [128, tet * K], f32, kind="ExternalInput")
    ewp_ext = nc.dram_tensor("ewp", [128, tet], f32, kind="ExternalInput")
    bitp_ext = nc.dram_tensor("bitp", [14, tet * 128], bf16, kind="ExternalInput")
    nbits_ext = nc.dram_tensor("nbits", [14, 128], bf16, kind="ExternalInput")
    featmy_ext = nc.dram_tensor("feat_my", [SH, D], f32, kind="ExternalInput")
    beta_ext = nc.dram_tensor("beta", [1, 1], f32, kind="ExternalInput")
    eps_ext = nc.dram_tensor("eps", [1, 1], f32, kind="ExternalInput")
    out_ext = nc.dram_tensor("out", [SH, D], f32, kind="ExternalOutput")

    GRP = 4          # dst tiles per norm group (ACT table loads amortized)
    ZCH = 96         # z pre-phase: edge tiles per chunk

    with tile.TileContext(nc) as tc:
        with (
            tc.tile_pool(name="persist", bufs=1) as pp,
            tc.tile_pool(name="msgp", bufs=GRP + 2) as mpool,
            tc.tile_pool(name="msgq", bufs=3) as mqpool,
            tc.tile_pool(name="sqp", bufs=3) as qpool,
            tc.tile_pool(name="zpre", bufs=2) as zpool,
            tc.tile_pool(name="small", bufs=2 * GRP + 2) as spool,
            tc.tile_pool(name="bitpool", bufs=GRP + 2) as bpool,
            tc.tile_pool(name="stp", bufs=4) as stpool,
            tc.tile_pool(name="outp", bufs=4) as opool,
            tc.tile_pool(name="mpsum", bufs=4, space="PSUM") as mpsum,
            tc.tile_pool(name="hpsum", bufs=2, space="PSUM") as hpsum,
        ):
            # ---------- scalars ----------
            beta_s = pp.tile([1, 1], f32, tag="beta_s")
            eps_s = pp.tile([1, 1], f32, tag="eps_s")
            nc.sync.dma_start(out=beta_s[:], in_=beta_ext[:])
            nc.sync.dma_start(out=eps_s[:], in_=eps_ext[:])
            beta_b = pp.tile([128, 1], f32, tag="beta_b")
            ep1_b = pp.tile([128, 1], f32, tag="ep1_b")
            nc.gpsimd.partition_broadcast(beta_b[:], beta_s[:])
            nc.gpsimd.partition_broadcast(ep1_b[:], eps_s[:])
            nc.vector.tensor_scalar_add(ep1_b[:], ep1_b[:], 1.0)
            neg6 = pp.tile([128, 1], f32, tag="neg6")
            nc.vector.memset(neg6[:], -6.0)
            sseps = pp.tile([128, 1], f32, tag="sseps")
            nc.vector.memset(sseps[:], 1e-12)

            # ---------- global per-edge weight w = exp(beta*ew) ----------
            wv = pp.tile([128, tet], f32, tag="wv")
            nc.sync.dma_start(out=wv[:], in_=ewp_ext[:])
            nc.scalar.activation(wv[:], wv[:], AF.Exp, scale=beta_b[:])

            nbits = pp.tile([14, 128], bf16, tag="nbits")
            nc.sync.dma_start(out=nbits[:], in_=nbits_ext[:])

            # ---------- z pre-phase: z = sum exp(beta*zrow), wz = w/z ------
            zs = pp.tile([128, tet], f32, tag="zs")
            for c0 in range(0, tet, ZCH):
                cw = min(ZCH, tet - c0)
                zrow = zpool.tile([128, ZCH, K], f32, tag="zrow")
                nc.sync.dma_start(
                    out=zrow[:, :cw, :].rearrange("p a b -> p (a b)"),
                    in_=ze_ext[:, c0 * K : (c0 + cw) * K],
                )
                zx = zpool.tile([128, ZCH, K], bf16, tag="zx")
                nc.scalar.activation(
                    zx[:, :cw, :].rearrange("p a b -> p (a b)"),
                    zrow[:, :cw, :].rearrange("p a b -> p (a b)"),
                    AF.Exp,
                    scale=beta_b[:],
                )
                nc.vector.tensor_reduce(
                    zs[:, c0 : c0 + cw], zx[:, :cw, :], X, ALU.add
                )
            wz = pp.tile([128, tet], f32, tag="wz")
            nc.vector.reciprocal(wz[:], zs[:])
            nc.vector.tensor_tensor(wz[:], wz[:], wv[:], ALU.mult)

            # ---------- main loop: groups of GRP dst tiles ----------
            offs = [0] * (HTILES + 1)
            for i in range(HTILES):
                offs[i + 1] = offs[i] + int(net[i])

            for i0 in range(0, HTILES, GRP):
                tiles = list(range(i0, min(i0 + GRP, HTILES)))
                msgs, sqs, sss, lnss, rrs, qs, bps = {}, {}, {}, {}, {}, {}, {}

                for i in tiles:
                    nh, T = int(net[i]), offs[i]
                    msg = mpool.tile([128, nmax, D], f32, tag="msg")
                    nc.sync.dma_start(
                        out=msg[:, :nh, :].rearrange("p a b -> p (a b)"),
                        in_=fe_ext[:, T * D : (T + nh) * D],
                    )
                    msgs[i] = msg
                    bp = bpool.tile([14, nmax, 128], bf16, tag="bp")
                    nc.sync.dma_start(
                        out=bp[:, :nh, :].rearrange("p a b -> p (a b)"),
                        in_=bitp_ext[:, T * 128 : (T + nh) * 128],
                    )
                    bps[i] = bp
                    sq = qpool.tile([128, nmax, D], bf16, tag="sq")
                    nc.gpsimd.tensor_tensor(
                        sq[:, :nh, :].rearrange("p a b -> p (a b)"),
                        msg[:, :nh, :].rearrange("p a b -> p (a b)"),
                        msg[:, :nh, :].rearrange("p a b -> p (a b)"),
                        ALU.mult,
                    )
                    sqs[i] = sq
                    ss = spool.tile([128, nmax], f32, tag="ss")
                    nc.vector.tensor_reduce(ss[:, :nh], sq[:, :nh, :], X, ALU.add)
                    sss[i] = ss

                # grouped ScalarE: all Lns, then all Exps (2 table loads/GRP)
                for i in tiles:
                    nh = int(net[i])
                    lns = spool.tile([128, nmax], f32, tag="lns")
                    nc.scalar.activation(
                        lns[:, :nh], sss[i][:, :nh], AF.Ln, bias=sseps[:]
                    )
                    lnss[i] = lns
                for i in tiles:
                    nh = int(net[i])
                    rr = spool.tile([128, nmax], f32, tag="rr")
                    nc.scalar.activation(
                        rr[:, :nh], lnss[i][:, :nh], AF.Exp, scale=-0.5
                    )
                    rrs[i] = rr
                for i in tiles:
                    nh, T = int(net[i]), offs[i]
                    q = spool.tile([128, nmax], f32, tag="q")
                    nc.vector.tensor_tensor(
                        q[:, :nh], rrs[i][:, :nh], wz[:, T : T + nh], ALU.mult
                    )
                    qs[i] = q

                for i in tiles:
                    nh, T = int(net[i]), offs[i]
                    nb4 = (nh + 3) // 4
                    rows = min(128, SH - i * 128)
                    msg, bp, q = msgs[i], bps[i], qs[i]

                    # q-scaled bf16 message rows (q broadcast along D)
                    msgq = mqpool.tile([128, nmax, D], bf16, tag="msgq")
                    qb = q[:, :nh].unsqueeze(2).broadcast_to([128, nh, D])
                    nc.vector.tensor_tensor(
                        msgq[:, :nh, :], msg[:, :nh, :], qb, ALU.mult
                    )

                    # edge tiles: batched one-hot build, mm1 one batch ahead
                    hp = hpsum.tile([128, D], f32, tag="hp")
                    st4s = [None] * nb4

                    def issue_batch(g):
                        t0 = 4 * g
                        w4 = min(4, nh - t0)
                        mp4 = mpsum.tile([128, 4, 128], f32, tag="mp4")
                        for j in range(w4):
                            nc.tensor.matmul(
                                mp4[:, j, :], bp[:, t0 + j, :], nbits[:],
                                start=True, stop=True,
                            )
                        st4 = stpool.tile([128, 4, 128], bf16, tag="st4")
                        sflat = st4[:, :w4, :].rearrange("p a b -> p (a b)")
                        mflat = mp4[:, :w4, :].rearrange("p a b -> p (a b)")
                        if g % 4 == 3:
                            nc.vector.tensor_scalar(
                                sflat, mflat, 7.0, None, op0=ALU.is_equal
                            )
                        else:
                            nc.scalar.activation(
                                sflat, mflat, AF.Relu, bias=neg6[:]
                            )
                        st4s[g] = st4

                    for g in range(nb4 + 1):
                        if g < nb4:
                            issue_batch(g)
                        gm = g - 1
                        if gm >= 0:
                            t0 = 4 * gm
                            for j in range(min(4, nh - t0)):
                                t = t0 + j
                                nc.tensor.matmul(
                                    hp[:],
                                    st4s[gm][:, j, :],
                                    msgq[:, t, :],
                                    start=(t == 0),
                                    stop=(t == nh - 1),
                                )

                    # out = hp + (1+eps)*feat_my
                    ftm = opool.tile([128, D], f32, tag="ftm")
                    nc.sync.dma_start(
                        out=ftm[:rows, :],
                        in_=featmy_ext[i * 128 : i * 128 + rows, :],
                    )
                    fts = opool.tile([128, D], f32, tag="fts")
                    nc.scalar.activation(
                        fts[:rows, :], ftm[:rows, :], AF.Copy,
                        scale=ep1_b[:rows, :],
                    )
                    ot = opool.tile([128, D], f32, tag="ot")
                    nc.vector.tensor_tensor(
                        ot[:rows, :], fts[:rows, :], hp[:rows, :], ALU.add
                    )
                    nc.sync.dma_start(
                        out=out_ext[i * 128 : i * 128 + rows, :], in_=ot[:rows, :]
                    )

    nc.finalize()
    return nc


def kernel(feat, edge_weight, src, dst, beta, eps):
    from concourse.bass_utils import run_bass_kernel_spmd

    feat = np.asarray(feat, dtype=np.float32)
    ew = np.asarray(edge_weight, dtype=np.float32)
    beta = np.asarray(beta, dtype=np.float32)
    eps = np.asarray(eps, dtype=np.float32)

    zpad, core_idx, nb, net, K = _host_prep(src, dst, ew)
    tet = int(net.sum())

    key = (K, tuple(int(x) for x in net))
    if key not in _COMPILED:
        _COMPILED[key] = _build(net, K)
    nc = _COMPILED[key]

    featP = np.vstack([feat, np.zeros((1, D), np.float32)])  # pad row = 0
    beta2 = beta.reshape(1, 1)
    eps2 = eps.reshape(1, 1)

    in_maps = []
    for c in range(NCORES):
        src_pad, ewd, bp = core_idx[c]
        fe = featP[src_pad].reshape(tet, 128, D).transpose(1, 0, 2)
        zE = zpad[src_pad].reshape(tet, 128, K).transpose(1, 0, 2)
        import ml_dtypes

        in_maps.append(
            {
                "feat_edges": np.ascontiguousarray(fe).reshape(128, tet * D),
                "zpadE": np.ascontiguousarray(zE).reshape(128, tet * K),
                "ewp": ewd,
                "bitp": np.ascontiguousarray(bp).astype(ml_dtypes.bfloat16),
                "nbits": nb,
                "feat_my": np.ascontiguousarray(feat[c * SH : (c + 1) * SH]),
                "beta": beta2,
                "eps": eps2,
            }
        )

    res = run_bass_kernel_spmd(nc, in_maps, core_ids=list(range(NCORES)))
    out = np.concatenate([res.results[c]["out"] for c in range(NCORES)], axis=0)
    return out.astype(np.float32)
